# revision 14
# baseline (speedup 1.0000x reference)
"""Nearest-neighbor tokenizer on Trainium2: 8 NeuronCores, code-sharded.

Per token x (d=512) against codebook C [16384, 512]:
    dist^2(x,c) = ||x||^2 + ||c||^2 - 2 x.c
    id = argmin_c dist^2   if min_c dist^2 <= 900 else -1

v10 architecture (bf16 score streaming, host-side selection).
Real-TRN2 engine constraints: GPSIMD does add/mult only (no max, no
PSUM); no instruction may read two PSUM operands; only ACT and DVE
touch PSUM; DMA cannot touch PSUM.
  - Shard by CODES: core g owns 2048 codes, sees all 8192 tokens.
    63 token tiles of 128 run on device; the last tile is cheaper to
    brute-force on the host than to pay the non-overlappable tail DMA
    latency for. Per tile, v = x.c - ||c||^2/2 lands in a 3-bank PSUM
    strip pss [128, 1536] + 1-bank psh [128, 512] via 8 fp8 DoubleRow
    matmuls (2 K=256 chunks x 4 banks). The code-norm bias is FOLDED
    into the 2nd chunk: contraction rows 508..511 hold a 4-term fp8
    decomposition of -||c||^2/2 (x-side rows = 1.0), so no separate
    bias matmul. The screen v drops dims 508..511 of the dot product
    (the host rescore is exact anyway).
  - PSUM drain split across the only two engines that can read PSUM,
    balanced per the cost model (ACT 0.83ns/el + 185ns/op, DVE
    1.04ns/el + 125ns/op, readers of one PSUM tile serialize):
      ACT: one bf16 Copy of pss[:, 0:1184] -> SBUF (1172 ns)
      DVE: max-reduce 32->1 of pss[:, 1184:1536] (ordered BEFORE the
        ACT copy so the serialized pss readers don't stall ACT) and of
        psh -> 11 + 16 bf16 group maxes (1150 ns)
    The idle sync queue streams the 1184 bf16 per-code scores to DRAM
    (913 ns/tile); gpsimd prefetches x tiles and flushes the group
    maxes. PE: 853 ns/tile. Steady state ~1172 ns/tile, ACT-bound.
  - Host: per token, group maxes of the streamed per-code scores ->
    64 group bounds per core; top-K groups over all 512; exact f64
    rescore of K*32 candidates; argmin + threshold. Exact as long as
    the global winner's group ranks in the top K (safety-checked in
    test.py; fp8 screen noise is ~14 max at TOPK margins ~40+).
"""

import sys

import numpy as np
import ml_dtypes

try:
    import concourse.bass as _probe_bass  # noqa: F401
except Exception:  # pragma: no cover
    sys.path.insert(0, "/opt/trn_rl_repo")

B, S, D = 4, 2048, 512
C = 16384
N_CORES = 8
NTOK = B * S                   # 8192 tokens, all seen by every core
N_TILES = NTOK // 128          # 64 token tiles (63 on device)
NDTILE = N_TILES - 1           # device tiles
G = C // N_CORES               # 2048 codes per core
KC2 = 2                        # DoubleRow contraction chunks (2 x 256)
NVB = 1184                     # per-code bf16 cols streamed out (37 groups)
NHA = 1536 - NVB               # DVE-reduced cols in pss (11 groups)
NHB = 512                      # DVE-reduced cols in psh (16 groups)
NVG = NVB // 32                # 37 streamed groups (host max)
NAG = NHA // 32                # 11
NBG = NHB // 32                # 16
NGRP = NAG + NBG               # 27 device group maxes per token per core
NGALL = NVG + NGRP             # 64 groups per core for host selection
TOPK = 16                      # host-side candidate groups per token
FP8 = ml_dtypes.float8_e4m3    # TRN fp8e4 (max normal 240)
BF16 = ml_dtypes.bfloat16

_CACHE: dict = {}


def _build_program(nc=None):
    import concourse.tile as tile
    from concourse import mybir

    f32 = mybir.dt.float32
    fp8 = mybir.dt.float8e4
    bf16 = mybir.dt.bfloat16
    Alu = mybir.AluOpType
    Act = mybir.ActivationFunctionType
    DR = mybir.MatmulPerfMode.DoubleRow

    if nc is None:
        # Bacc: its finalize() runs the TRN2 wait-splitting compile passes
        # (plain Bass emits multi-wait DMAs that walrus codegen rejects).
        from concourse import bacc

        nc = bacc.Bacc("TRN2", target_bir_lowering=False, debug=False)

    xs_d = nc.declare_dram_parameter("xs", [128, N_TILES, KC2, 2, 128], fp8, isOutput=False)
    cr_d = nc.declare_dram_parameter("cr", [128, KC2, 4, 2, 512], fp8, isOutput=False)
    sc_d = nc.declare_dram_parameter("sc", [128, (NDTILE + 7) // 8, 8, NGRP], bf16, isOutput=True)
    vb_d = nc.declare_dram_parameter("vb", [128, NDTILE, NVB], bf16, isOutput=True)

    with tile.TileContext(nc) as tc:
        with (
            tc.tile_pool(name="const", bufs=1) as const,
            tc.tile_pool(name="work", bufs=3) as work,
            tc.tile_pool(name="scout", bufs=2) as scout,
            tc.tile_pool(name="psum", bufs=2, space="PSUM") as psum,
        ):
            crb = const.tile([128, KC2, 4, 2, 512], fp8, name="crb")
            xsb = const.tile([128, N_TILES, KC2, 2, 128], fp8, name="xsb")
            # 3-lane fill (sync/gpsimd/scalar all issue DMAs; scalar is
            # idle until the first ACT copy at ~4us)
            nc.scalar.dma_start(xsb[:, 0:2], xs_d[:, 0:2])
            nc.gpsimd.dma_start(crb[:, :, 3:4], cr_d[:, :, 3:4])
            nc.sync.dma_start(crb[:, :, 0:2], cr_d[:, :, 0:2])
            nc.gpsimd.dma_start(crb[:, :, 2:3], cr_d[:, :, 2:3])
            nc.sync.dma_start(xsb[:, 2:4], xs_d[:, 2:4])
            nc.gpsimd.dma_start(xsb[:, 4:8], xs_d[:, 4:8])
            for i, t0 in enumerate(range(8, N_TILES, 8)):
                nc.gpsimd.dma_start(xsb[:, t0:t0 + 8], xs_d[:, t0:t0 + 8])
            # warm the PE p-state ramp: dummy matmuls on a zeroed tile keep
            # the tensor engine busy until the first real matmuls
            wx = const.tile([128, 2, 128], fp8, name="wx")
            nc.vector.memset(wx[:], 0.0)
            wps = psum.tile([128, NVB + NHA], f32, name="pss")
            for i in range(36):
                nc.tensor.matmul(wps[:, 0:128], wx[:], wx[:],
                                 start=(i == 0), stop=(i == 35), perf_mode=DR)

            sco = None
            for t in range(NDTILE):
                k = t % 8
                if k == 0:
                    sco = scout.tile([128, 8, NGRP], bf16, name="sco")
                pss = psum.tile([128, NVB + NHA], f32, name="pss")
                psh = psum.tile([128, NHB], f32, name="psh")
                for c in range(KC2):
                    nc.tensor.matmul(
                        psh[:], xsb[:, t, c], crb[:, c, 3],
                        start=(c == 0), stop=(c == KC2 - 1), perf_mode=DR,
                    )
                for b in range(3):
                    for c in range(KC2):
                        nc.tensor.matmul(
                            pss[:, b * 512:(b + 1) * 512],
                            xsb[:, t, c], crb[:, c, b],
                            start=(c == 0), stop=(c == KC2 - 1),
                            perf_mode=DR,
                        )
                # DVE reduces FIRST (pss readers serialize in program
                # order; the short reduce must not stall the long copy)
                nc.vector.tensor_reduce(
                    sco[:, k, 0:NAG],
                    pss[:, NVB:].rearrange("p (u l) -> p u l", u=NAG),
                    axis=mybir.AxisListType.X, op=Alu.max,
                )
                nc.vector.tensor_reduce(
                    sco[:, k, NAG:NGRP],
                    psh[:].rearrange("p (u l) -> p u l", u=NBG),
                    axis=mybir.AxisListType.X, op=Alu.max,
                )
                # ACT: bulk bf16 copy of the per-code scores to SBUF
                vb = work.tile([128, NVB], bf16, name="vb")
                nc.scalar.activation(vb[:], pss[:, 0:NVB], Act.Copy)
                # idle sync lane streams them to DRAM
                nc.sync.dma_start(vb_d[:, t], vb[:])
                if k == 7:
                    nc.gpsimd.dma_start(sc_d[:, t // 8], sco[:])
                elif t == NDTILE - 1:
                    nc.gpsimd.dma_start(sc_d[:, t // 8, 0:7], sco[:, 0:7])

    return nc


def _fp8r(a):
    return np.asarray(a, np.float32).astype(FP8)


def _prepare_in_maps(x: np.ndarray, codes: np.ndarray) -> list:
    x = np.ascontiguousarray(np.asarray(x, dtype=np.float32).reshape(NTOK, D))
    codes = np.ascontiguousarray(np.asarray(codes, dtype=np.float32))

    # xs[p, t, c, i, m] = fp8(x)[t*128 + m, c*256 + i*128 + p]  (all cores)
    xq = _fp8r(x)
    xs = np.ascontiguousarray(
        xq.reshape(N_TILES, 128, KC2, 2, 128).transpose(4, 0, 2, 3, 1)
    )
    # bias rows: contraction rows 508..511 (c=1, i=1, p=124..127) carry the
    # code-norm terms; x side is 1.0 there (dims 508..511 leave the screen)
    xs[124:128, :, 1, 1, :] = np.float32(1.0).astype(FP8)

    in_maps = []
    for g in range(N_CORES):
        cg = codes[g * G:(g + 1) * G]  # [2048, 512]
        cq = _fp8r(cg)
        # cr[p, c, b, i, n] = fp8(cg)[b*512 + n, c*256 + i*128 + p]
        cr = np.ascontiguousarray(
            cq.reshape(4, 512, KC2, 2, 128).transpose(4, 2, 0, 3, 1)
        )
        # 4-term fp8 decomposition of bias = -||c||^2/2 (|bias| ~ 256
        # exceeds fp8e4m3 max 240, so split b/2 + b/2 + resid + resid)
        bias = -0.5 * (cg.astype(np.float64) ** 2).sum(1)
        t1 = _fp8r(bias * 0.5)
        t2 = t1.copy()
        r = bias - t1.astype(np.float64) - t2.astype(np.float64)
        t3 = _fp8r(r)
        t4 = _fp8r(r - t3.astype(np.float64))
        T = np.stack([t1, t2, t3, t4]).astype(FP8)  # [4, 2048]
        cr[124:128, 1, :, 1, :] = T.reshape(4, 4, 512)
        in_maps.append({"xs": xs, "cr": cr})
    return in_maps


def _select_candidates(results: list) -> np.ndarray:
    """Per-token TOPK candidate groups from the streamed scores.

    Returns cand_codes [NDEV, TOPK*32] int64 (code ids, may repeat)."""
    NDEV = NTOK - 128  # tokens of the 63 device tiles; tile 63 is host-side
    U = np.empty((NDEV, N_CORES * NGALL), np.float32)
    for g in range(N_CORES):
        # vb[p, t, j]: token = t*128 + p; per-code bf16 v-hat
        vbr = np.asarray(results[g]["vb"]).astype(np.float32)
        uvb = vbr.transpose(1, 0, 2).reshape(NDEV, NVG, 32).max(2)
        sc = np.asarray(results[g]["sc"]).astype(np.float32)  # [128,8,8,27]
        sc = sc.transpose(1, 2, 0, 3).reshape(NTOK, NGRP)[:NDEV]
        U[:, g * NGALL:(g + 1) * NGALL] = np.concatenate([uvb, sc], axis=1)
    topg = np.argpartition(-U, TOPK, axis=1)[:, :TOPK]  # [NDEV, TOPK]
    core = topg // NGALL
    j = topg % NGALL
    base = j * 32  # identity layout: streamed | pss-tail | psh, 32s
    code0 = core * G + base  # [NDEV, TOPK]
    cands = (code0[:, :, None] + np.arange(32)[None, None, :]).reshape(NDEV, TOPK * 32)
    return cands


def _postprocess(results: list, x: np.ndarray, codes: np.ndarray) -> np.ndarray:
    x64 = np.asarray(x, dtype=np.float64).reshape(NTOK, D)
    c64 = np.asarray(codes, dtype=np.float64)
    c2 = (c64 ** 2).sum(1)
    x2 = (x64 ** 2).sum(1)

    cands = _select_candidates(results)
    cands.sort(axis=1)  # argmin tie-break: first occurrence = lowest index

    NDEV = cands.shape[0]
    ids = np.empty(NTOK, np.int64)
    CH = 64
    rows = np.arange(CH)
    for i in range(0, NDEV, CH):
        cc = cands[i:i + CH]
        xc = np.einsum("tkd,td->tk", c64[cc], x64[i:i + CH], optimize=True)
        d2 = np.maximum(x2[i:i + CH, None] + c2[cc] - 2.0 * xc, 0.0)
        kk = d2.argmin(1)
        ids[i:i + CH] = np.where(d2[rows, kk] <= 900.0, cc[rows, kk], -1)
    # tile 63 never leaves the device: exact brute force over all codes
    for i in range(NDEV, NTOK, CH):
        d2 = np.maximum(
            x2[i:i + CH, None] + c2[None, :] - 2.0 * (x64[i:i + CH] @ c64.T), 0.0
        )
        kk = d2.argmin(1)
        ids[i:i + CH] = np.where(d2[rows, kk] <= 900.0, kk, -1)
    return ids.reshape(B, S).astype(np.int32)


def kernel(x: np.ndarray, codes: np.ndarray) -> np.ndarray:
    from concourse.bass_utils import run_bass_kernel_spmd

    if "nc" not in _CACHE:
        nc = _build_program()
        nc.finalize()  # Bacc: runs wait-splitting + register allocation
        _CACHE["nc"] = nc
    in_maps = _prepare_in_maps(x, codes)
    res = run_bass_kernel_spmd(_CACHE["nc"], in_maps, list(range(N_CORES)))
    return _postprocess(res.results, x, codes)
